# revision 8
# baseline (speedup 1.0000x reference)
"""AdaptiveStateMixer kernel for 8 Trainium2 NeuronCores.

Strategy:
  - Data parallel over batch: B=8 batches -> one batch per core, no collectives.
  - Host precomputes: transposed states XT/MT ([H,S]), and a combined
    additive bias+mask matrix per batch ([S,S]) holding
    order_bias + time_bias, with -1e30 at invalid (causal/padded) slots.
  - On chip (per core): qT = (Wq.T@XT + bq)/sqrt(H), kT = Wk.T@MT + bk,
    v = gelu(MT.T@Wv + bv); scores = qT.T@kT + bias; softmax along free dim
    (rows with no valid key get zeroed via a row-max flag); attn written to
    DRAM; attn transposed via PE transpose; ret = gelu((attn @ v) @ Wo + bo).
  - All matmuls run in float32r (full-rate fp32, ~1e-4 rel err).
  - Strictly causal: score/attn-v tiles entirely above the diagonal are
    skipped; the attn output relies on the runtime's pre-zeroed output
    buffers for the never-written upper triangle.
"""

import os
import sys

sys.path.insert(0, "/opt/trn_rl_repo")

import numpy as np

B, S, H, P = 8, 2048, 1024, 128
NEG = np.float32(-1e30)
HC = H // P     # 8 h-chunks
DC = H // P     # 8 d-chunks
NB = S // 512   # 4 row blocks of 512
N_CORES = 8

_RUNNER = None  # (run_fn, build_key)


def _build_nc(nonzero_bq, nonzero_bk, nonzero_bv, nonzero_bo):
    import concourse.bass as bass
    import concourse.mybir as mybir
    import concourse.tile as tile
    from concourse import bacc
    from concourse.masks import make_identity

    dt = mybir.dt
    AF = mybir.ActivationFunctionType
    GELU = AF.Identity if os.environ.get("K_NOGELU") else AF.Gelu
    OP = mybir.AluOpType
    AX = mybir.AxisListType
    PHASE = int(os.environ.get("K_PHASE", "9"))

    nc = bacc.Bacc(None, target_bir_lowering=False)

    xt = nc.declare_dram_parameter("xt", [H, S], dt.float32r, isOutput=False)
    mt = nc.declare_dram_parameter("mt", [H, S], dt.float32r, isOutput=False)
    wq = nc.declare_dram_parameter("wq", [H, H], dt.float32r, isOutput=False)
    wk = nc.declare_dram_parameter("wk", [H, H], dt.float32r, isOutput=False)
    wv = nc.declare_dram_parameter("wv", [H, H], dt.float32r, isOutput=False)
    wo = nc.declare_dram_parameter("wo", [H, H], dt.float32r, isOutput=False)
    bias = nc.declare_dram_parameter("bias", [S, S], dt.float32, isOutput=False)
    bqp = bkp = bvp = bop = None
    if nonzero_bq:
        bqp = nc.declare_dram_parameter("bq", [P, HC], dt.float32, isOutput=False)
    if nonzero_bk:
        bkp = nc.declare_dram_parameter("bk", [P, HC], dt.float32, isOutput=False)
    if nonzero_bv:
        bvp = nc.declare_dram_parameter("bv", [P, H], dt.float32, isOutput=False)
    if nonzero_bo:
        bop = nc.declare_dram_parameter("bo", [P, H], dt.float32, isOutput=False)
    attn_out = nc.declare_dram_parameter("attn", [S, S], dt.float32, isOutput=True)
    ret_out = nc.declare_dram_parameter("ret", [S, H], dt.float32, isOutput=True)

    qts = nc.dram_tensor("qts", [H, S], dt.float32r)  # qT scratch

    inv_sqrt_h = 1.0 / float(np.sqrt(H))

    with tile.TileContext(nc) as tc:
        with (
            tc.tile_pool(name="const", bufs=1) as cpool,
            tc.tile_pool(name="kt", bufs=HC) as kt_pool,
            tc.tile_pool(name="v", bufs=S // P) as v_pool,
            tc.tile_pool(name="psum", bufs=8, space="PSUM") as pp,
        ):
            ident = cpool.tile([P, P], dt.float32)
            make_identity(nc, ident[:])
            bq_sb = bk_sb = bv_sb = bo_sb = None
            if bqp is not None:
                bq_sb = cpool.tile([P, HC], dt.float32)
                nc.sync.dma_start(bq_sb[:], bqp[:])
            if bkp is not None:
                bk_sb = cpool.tile([P, HC], dt.float32)
                nc.sync.dma_start(bk_sb[:], bkp[:])
            if bvp is not None:
                bv_sb = cpool.tile([P, H], dt.float32)
                nc.sync.dma_start(bv_sb[:], bvp[:])
            if bop is not None:
                bo_sb = cpool.tile([P, H], dt.float32)
                nc.sync.dma_start(bo_sb[:], bop[:])

            kt_sb = [kt_pool.tile([P, S], dt.float32r, tag="kt", name="kt") for _ in range(HC)]
            v_sb = [v_pool.tile([P, H], dt.float32r, tag="v", name="v") for _ in range(S // P)]

            # ---------------- Phase 0: projections ----------------
            with (
                tc.tile_pool(name="w", bufs=8) as wpool,
                tc.tile_pool(name="xm", bufs=12) as xmpool,
                tc.tile_pool(name="xmv", bufs=16) as xmvpool,
                tc.tile_pool(name="qo", bufs=4) as qopool,
                tc.tile_pool(name="tmp0", bufs=4) as tmp0pool,
            ):
                # qT = (Wq.T @ XT + bq) * inv_sqrt_h  -> qts
                wq_t = []
                for c in range(DC):
                    t = wpool.tile([P, H], dt.float32r, tag="w", name="w")
                    nc.sync.dma_start(t[:], wq[c * P:(c + 1) * P, :])
                    wq_t.append(t)
                for ic in range(S // 512):
                    xt_t = []
                    for c in range(DC):
                        t = xmpool.tile([P, 512], dt.float32r, tag="xm", name="xm")
                        nc.sync.dma_start(
                            t[:], xt[c * P:(c + 1) * P, ic * 512:(ic + 1) * 512]
                        )
                        xt_t.append(t)
                    for hb in range(HC):
                        ps = pp.tile([P, 512], dt.float32, tag="ps", name="ps")
                        for c in range(DC):
                            nc.tensor.matmul(
                                ps[:],
                                wq_t[c][:, hb * P:(hb + 1) * P],
                                xt_t[c][:],
                                start=(c == 0),
                                stop=(c == DC - 1),
                            )
                        qsb = qopool.tile([P, 512], dt.float32r, tag="qo", name="qo")
                        nc.vector.tensor_scalar(
                            qsb[:],
                            ps[:],
                            bq_sb[:, hb:hb + 1] if bq_sb is not None else 0.0,
                            inv_sqrt_h,
                            op0=OP.add,
                            op1=OP.mult,
                        )
                        nc.sync.dma_start(
                            qts[hb * P:(hb + 1) * P, ic * 512:(ic + 1) * 512], qsb[:]
                        )

                # kT = Wk.T @ MT + bk  -> resident
                wk_t = []
                for c in range(DC):
                    t = wpool.tile([P, H], dt.float32r, tag="w", name="w")
                    nc.sync.dma_start(t[:], wk[c * P:(c + 1) * P, :])
                    wk_t.append(t)
                for jc4 in range(S // 512):
                    mt_t = []
                    for c in range(DC):
                        t = xmpool.tile([P, 512], dt.float32r, tag="xm", name="xm")
                        nc.sync.dma_start(
                            t[:], mt[c * P:(c + 1) * P, jc4 * 512:(jc4 + 1) * 512]
                        )
                        mt_t.append(t)
                    for hb in range(HC):
                        ps = pp.tile([P, 512], dt.float32, tag="ps", name="ps")
                        for c in range(DC):
                            nc.tensor.matmul(
                                ps[:],
                                wk_t[c][:, hb * P:(hb + 1) * P],
                                mt_t[c][:],
                                start=(c == 0),
                                stop=(c == DC - 1),
                            )
                        nc.vector.tensor_scalar(
                            kt_sb[hb][:, jc4 * 512:(jc4 + 1) * 512],
                            ps[:],
                            bk_sb[:, hb:hb + 1] if bk_sb is not None else 0.0,
                            None,
                            op0=OP.add,
                        )

                # v = gelu(MT.T @ Wv + bv) -> resident
                wv_t = []
                for c in range(DC):
                    t = wpool.tile([P, H], dt.float32r, tag="w", name="w")
                    nc.sync.dma_start(t[:], wv[c * P:(c + 1) * P, :])
                    wv_t.append(t)
                for jb in range(S // P):
                    mtv_t = []
                    for c in range(DC):
                        t = xmvpool.tile([P, P], dt.float32r, tag="xmv", name="xmv")
                        nc.sync.dma_start(
                            t[:], mt[c * P:(c + 1) * P, jb * P:(jb + 1) * P]
                        )
                        mtv_t.append(t)
                    for hcol in range(H // 512):
                        ps = pp.tile([P, 512], dt.float32, tag="ps", name="ps")
                        for c in range(DC):
                            nc.tensor.matmul(
                                ps[:],
                                mtv_t[c][:],
                                wv_t[c][:, hcol * 512:(hcol + 1) * 512],
                                start=(c == 0),
                                stop=(c == DC - 1),
                            )
                        dst = v_sb[jb][:, hcol * 512:(hcol + 1) * 512]
                        if bv_sb is not None:
                            tmp = tmp0pool.tile([P, 512], dt.float32, tag="tmp0", name="tmp0")
                            nc.vector.tensor_tensor(
                                tmp[:], ps[:],
                                bv_sb[:, hcol * 512:(hcol + 1) * 512], OP.add,
                            )
                            nc.scalar.activation(dst, tmp[:], GELU)
                        else:
                            nc.scalar.activation(dst, ps[:], GELU)

            # ---------------- Phase 1: attention ----------------
            with (
                tc.tile_pool(name="at", bufs=16) as atpool,
                tc.tile_pool(name="qt", bufs=8) as qtpool,
                tc.tile_pool(name="s", bufs=2) as spool,
                tc.tile_pool(name="rt", bufs=8) as rtpool,
                tc.tile_pool(name="wo", bufs=3) as wopool,
                tc.tile_pool(name="og", bufs=2) as ogpool,
                tc.tile_pool(name="st", bufs=3) as stpool,
            ):
                for b4 in range(NB if PHASE >= 1 else 0):
                    Jb = 512 * (b4 + 1)
                    njt = b4 + 1
                    njc = 4 * (b4 + 1)
                    at = [
                        atpool.tile([P, 512], dt.float32r, tag="at", name="at")
                        for _ in range(njc)
                    ]
                    s_tiles = {}

                    def emit_scores(bi):
                        qt_t = []
                        for hcn in range(HC):
                            t = qtpool.tile([P, P], dt.float32r, tag="qt", name="qt")
                            nc.sync.dma_start(
                                t[:],
                                qts[hcn * P:(hcn + 1) * P, bi * P:(bi + 1) * P],
                            )
                            qt_t.append(t)
                        Sx = spool.tile([P, S], dt.float32, tag="s", name="s")
                        s_tiles[bi] = Sx
                        nc.sync.dma_start(
                            Sx[:, :Jb], bias[bi * P:(bi + 1) * P, :Jb]
                        )
                        mx4 = stpool.tile([P, 4], dt.float32, tag="mx4", name="mx4")
                        for jt in range(njt):
                            ps = pp.tile([P, 512], dt.float32, tag="ps", name="ps")
                            for hcn in range(HC):
                                nc.tensor.matmul(
                                    ps[:],
                                    qt_t[hcn][:],
                                    kt_sb[hcn][:, jt * 512:(jt + 1) * 512],
                                    start=(hcn == 0),
                                    stop=(hcn == HC - 1),
                                )
                            nc.vector.tensor_tensor(
                                Sx[:, jt * 512:(jt + 1) * 512],
                                ps[:],
                                Sx[:, jt * 512:(jt + 1) * 512],
                                OP.add,
                            )
                            nc.vector.reduce_max(
                                mx4[:, jt:jt + 1],
                                Sx[:, jt * 512:(jt + 1) * 512],
                                axis=AX.X,
                            )
                        nmx = stpool.tile([P, 1], dt.float32, tag="nmx", name="nmx")
                        nc.vector.reduce_max(
                            nmx[:], mx4[:, :njt], axis=AX.X, negate=True
                        )
                        sume = stpool.tile([P, 1], dt.float32, tag="sume", name="sume")
                        nc.scalar.activation(
                            Sx[:, :Jb], Sx[:, :Jb], AF.Exp,
                            bias=nmx[:], accum_out=sume[:],
                        )
                        rcp = stpool.tile([P, 1], dt.float32, tag="rcp", name="rcp")
                        nc.vector.reciprocal(rcp[:], sume[:])
                        flag = stpool.tile([P, 1], dt.float32, tag="flag", name="flag")
                        nc.vector.tensor_scalar(
                            flag[:], nmx[:], 1.0e29, None, op0=OP.is_lt
                        )
                        rcpz = stpool.tile([P, 1], dt.float32, tag="rcpz", name="rcpz")
                        nc.vector.tensor_tensor(rcpz[:], rcp[:], flag[:], OP.mult)
                        nc.vector.tensor_scalar(
                            Sx[:, :Jb], Sx[:, :Jb], rcpz[:], None, op0=OP.mult
                        )
                        nc.sync.dma_start(
                            attn_out[bi * P:(bi + 1) * P, :Jb], Sx[:, :Jb]
                        )

                    def emit_transposes(bi):
                        Sx = s_tiles[bi]
                        col = (bi % 4) * P
                        for g in range(njt):
                            pst = pp.tile([P, 512], dt.float32, tag="ps", name="ps")
                            for k in range(4):
                                nc.tensor.transpose(
                                    pst[:, k * P:(k + 1) * P],
                                    Sx[:, (4 * g + k) * P:(4 * g + k + 1) * P],
                                    ident[:],
                                )
                            for k in range(4):
                                nc.any.tensor_copy(
                                    out=at[4 * g + k][:, col:col + P],
                                    in_=pst[:, k * P:(k + 1) * P],
                                )

                    prev = None
                    for bi in range(4 * b4, 4 * b4 + 4):
                        emit_scores(bi)
                        if prev is not None and PHASE >= 2:
                            emit_transposes(prev)
                        prev = bi
                    if PHASE >= 2:
                        emit_transposes(prev)
                    if PHASE < 3:
                        continue

                    # ret_preT[h, i] = sum_j v[j, h] * attnT[j, i]
                    rt = [
                        rtpool.tile([P, 512], dt.float32r, tag="rt", name="rt")
                        for _ in range(HC)
                    ]
                    for hb in range(HC):
                        ps = pp.tile([P, 512], dt.float32, tag="ps", name="ps")
                        for jc in range(njc):
                            nc.tensor.matmul(
                                ps[:],
                                v_sb[jc][:, hb * P:(hb + 1) * P],
                                at[jc][:],
                                start=(jc == 0),
                                stop=(jc == njc - 1),
                            )
                        nc.any.tensor_copy(out=rt[hb][:], in_=ps[:])

                    # retrieved = gelu(ret_pre @ Wo + bo)
                    for gcol in range(H // 512):
                        ps_o = [
                            pp.tile([P, 512], dt.float32, tag="ps", name="ps") for _ in range(4)
                        ]
                        for hb in range(HC):
                            wo_t = wopool.tile([P, 512], dt.float32r, tag="wo", name="wo")
                            nc.sync.dma_start(
                                wo_t[:],
                                wo[hb * P:(hb + 1) * P,
                                   gcol * 512:(gcol + 1) * 512],
                            )
                            for isub in range(4):
                                nc.tensor.matmul(
                                    ps_o[isub][:],
                                    rt[hb][:, isub * P:(isub + 1) * P],
                                    wo_t[:],
                                    start=(hb == 0),
                                    stop=(hb == HC - 1),
                                )
                        for isub in range(4):
                            og = ogpool.tile([P, 512], dt.float32, tag="og", name="og")
                            if bo_sb is not None:
                                nc.vector.tensor_tensor(
                                    og[:], ps_o[isub][:],
                                    bo_sb[:, gcol * 512:(gcol + 1) * 512], OP.add,
                                )
                                nc.scalar.activation(og[:], og[:], GELU)
                            else:
                                nc.scalar.activation(og[:], ps_o[isub][:], GELU)
                            row = (4 * b4 + isub) * P
                            nc.sync.dma_start(
                                ret_out[row:row + P, gcol * 512:(gcol + 1) * 512],
                                og[:],
                            )

    nc.finalize()
    return nc


def _make_runner(nc):
    """Build a cached multi-core PJRT runner for a finalized Bass program.

    Mirrors concourse.bass2jax.run_bass_via_pjrt's multi-core path, but keeps
    the jitted callable so repeated invocations don't re-trace/re-compile.
    """
    import jax
    import jax.numpy as jnp  # noqa: F401
    import concourse.mybir as mybir
    from jax.experimental.shard_map import shard_map
    from jax.sharding import Mesh, PartitionSpec
    from concourse import bass2jax as b2j

    b2j.install_neuronx_cc_hook()

    partition_name = (
        nc.partition_id_tensor.name if nc.partition_id_tensor else None
    )
    in_names, out_names, out_avals, zero_shapes = [], [], [], []
    for alloc in nc.m.functions[0].allocations:
        if not isinstance(alloc, mybir.MemoryLocationSet):
            continue
        name = alloc.memorylocations[0].name
        if alloc.kind == "ExternalInput":
            if name != partition_name:
                in_names.append(name)
        elif alloc.kind == "ExternalOutput":
            shape = tuple(alloc.tensor_shape)
            dtype = mybir.dt.np(alloc.dtype)
            out_names.append(name)
            out_avals.append(jax.core.ShapedArray(shape, dtype))
            zero_shapes.append((shape, dtype))
    n_params = len(in_names)
    n_outs = len(out_avals)
    all_in_names = list(in_names) + list(out_names)
    if partition_name is not None:
        all_in_names.append(partition_name)
    donate = tuple(range(n_params, n_params + n_outs))

    def _body(*args):
        operands = list(args)
        if partition_name is not None:
            operands.append(b2j.partition_id_tensor())
        outs = b2j._bass_exec_p.bind(
            *operands,
            out_avals=tuple(out_avals),
            in_names=tuple(all_in_names),
            out_names=tuple(out_names),
            lowering_input_output_aliases=(),
            sim_require_finite=True,
            sim_require_nnan=True,
            nc=nc,
        )
        return tuple(outs)

    devices = jax.devices()[:N_CORES]
    mesh = Mesh(np.asarray(devices), ("core",))
    in_specs = (PartitionSpec("core"),) * (n_params + n_outs)
    out_specs = (PartitionSpec("core"),) * n_outs
    sharded = jax.jit(
        shard_map(
            _body, mesh=mesh, in_specs=in_specs, out_specs=out_specs,
            check_rep=False,
        ),
        donate_argnums=donate,
        keep_unused=True,
    )

    def make_zeros():
        return [
            np.zeros((N_CORES * shp[0], *shp[1:]), dtp)
            for shp, dtp in zero_shapes
        ]

    def concat_inputs(in_maps):
        return [
            np.concatenate([np.asarray(in_maps[c][k]) for c in range(N_CORES)],
                           axis=0)
            for k in in_names
        ]

    def run(in_maps):
        out_arrs = sharded(*concat_inputs(in_maps), *make_zeros())
        return [
            {
                k: np.asarray(out_arrs[i]).reshape(
                    N_CORES, *out_avals[i].shape
                )[c]
                for i, k in enumerate(out_names)
            }
            for c in range(N_CORES)
        ]

    run.sharded = sharded
    run.concat_inputs = concat_inputs
    run.make_zeros = make_zeros
    run.out_names = out_names
    run.out_avals = out_avals
    return run


def _prepare_in_maps(query_state, memory_state, time_seq, Wq, bq, Wk, bk,
                     Wv, bv, Wo, bo, order_scale, time_scale, padding_mask):
    q = np.asarray(query_state, dtype=np.float32)
    m = np.asarray(memory_state, dtype=np.float32)
    t = np.asarray(time_seq, dtype=np.float32)
    pad = np.asarray(padding_mask).astype(bool)
    Wq = np.ascontiguousarray(np.asarray(Wq, dtype=np.float32))
    Wk = np.ascontiguousarray(np.asarray(Wk, dtype=np.float32))
    Wv = np.ascontiguousarray(np.asarray(Wv, dtype=np.float32))
    Wo = np.ascontiguousarray(np.asarray(Wo, dtype=np.float32))
    bq = np.asarray(bq, dtype=np.float32)
    bk = np.asarray(bk, dtype=np.float32)
    bv = np.asarray(bv, dtype=np.float32)
    bo = np.asarray(bo, dtype=np.float32)

    def softplus(x):
        x = float(np.asarray(x))
        return float(np.log1p(np.exp(-abs(x))) + max(x, 0.0))

    co = np.float32(-softplus(order_scale))
    ct = np.float32(-softplus(time_scale))

    idx = np.arange(S, dtype=np.float32)
    od = np.maximum(idx[:, None] - idx[None, :], 0.0)
    ob = co * np.log1p(od)  # [S,S] f32
    tri = np.triu(np.ones((S, S), dtype=bool))  # strictly-causal invalid

    flags = (bool(np.any(bq)), bool(np.any(bk)),
             bool(np.any(bv)), bool(np.any(bo)))

    in_maps = []
    for b in range(B):
        tg = np.maximum(t[b][:, None] - t[b][None, :], 0.0)
        bias_b = ob + ct * np.log1p(tg)
        invalid = tri | pad[b][None, :]
        bias_b[invalid] = NEG
        im = {
            "xt": np.ascontiguousarray(q[b].T),
            "mt": np.ascontiguousarray(m[b].T),
            "wq": Wq, "wk": Wk, "wv": Wv, "wo": Wo,
            "bias": np.ascontiguousarray(bias_b, dtype=np.float32),
        }
        if flags[0]:
            im["bq"] = np.ascontiguousarray(bq.reshape(HC, P).T)
        if flags[1]:
            im["bk"] = np.ascontiguousarray(bk.reshape(HC, P).T)
        if flags[2]:
            im["bv"] = np.ascontiguousarray(np.broadcast_to(bv, (P, H)))
        if flags[3]:
            im["bo"] = np.ascontiguousarray(np.broadcast_to(bo, (P, H)))
        in_maps.append(im)
    return in_maps, flags


def _get_runner(flags):
    global _RUNNER
    if _RUNNER is None or _RUNNER[1] != flags:
        nc = _build_nc(*flags)
        _RUNNER = (_make_runner(nc), flags)
    return _RUNNER[0]


def kernel(**inputs):
    in_maps, flags = _prepare_in_maps(**inputs)
    run = _get_runner(flags)
    res = run(in_maps)
    retrieved = np.stack([res[c]["ret"] for c in range(N_CORES)])
    attn = np.stack([res[c]["attn"] for c in range(N_CORES)])
    return retrieved, attn


# revision 17
# speedup vs baseline: 27.1407x; 27.1407x over previous
"""AdaptiveStateMixer kernel for 8 Trainium2 NeuronCores.

Strategy:
  - Data parallel over batch: B=8 batches -> one batch per core, no collectives.
  - Host precomputes: transposed states XT/MT ([H,S]), a block-transposed copy
    of MT (for large contiguous DMA loads in the value projection), and a
    combined additive bias+mask matrix per batch ([S,S]) holding
    order_bias + time_bias, with -1e30 at invalid (causal/padded) slots.
  - On chip (per core): qT = (Wq.T@XT + bq)/sqrt(H) (roundtripped through a
    blocked DRAM scratch), kT = Wk.T@MT + bk (SBUF-resident),
    v = gelu(MT.T@Wv + bv); scores = qT.T@kT + bias; softmax along the free
    dim (rows with no valid key get zeroed via a row-max flag); attn written
    to DRAM; attn transposed via PE transpose; ret = gelu((attn @ v) @ Wo+bo).
  - The q/k/score path runs in float32r (full-rate fp32 matmul, ~1e-4 err);
    the post-softmax value path (v, attnT, ret_pre, Wo) runs in bf16.
  - Strictly causal: score/attn-v tiles entirely above the diagonal are
    skipped; the attn output relies on the runtime's pre-zeroed output
    buffers for the never-written upper triangle.
"""

import os
import sys

sys.path.insert(0, "/opt/trn_rl_repo")

import numpy as np

B, S, H, P = 8, 2048, 1024, 128
NEG = np.float32(-1e30)
HC = H // P     # 8 h-chunks
DC = H // P     # 8 d-chunks
NB = S // 512   # 4 row blocks of 512
NJB = S // P    # 16 column blocks of 128
N_CORES = 8

_RUNNER = None  # (run_fn, build_key)


def _build_nc(nonzero_bq, nonzero_bk, nonzero_bv, nonzero_bo):
    import concourse.mybir as mybir
    import concourse.tile as tile
    from concourse import bacc
    from concourse.masks import make_identity

    dt = mybir.dt
    AF = mybir.ActivationFunctionType
    GELU = AF.Identity if os.environ.get("K_NOGELU") else AF.Gelu
    OP = mybir.AluOpType
    AX = mybir.AxisListType
    PHASE = int(os.environ.get("K_PHASE", "9"))
    NODEFER = not bool(os.environ.get("K_DEFER"))
    F32OP = bool(os.environ.get("K_F32OP"))
    F32RET = bool(os.environ.get("K_F32RET"))
    BF = dt.float32r if F32RET else dt.bfloat16
    BFO = dt.float32r if F32OP else dt.bfloat16

    nc = bacc.Bacc(None, target_bir_lowering=False)

    xt = nc.declare_dram_parameter("xt", [H, S], dt.float32r, isOutput=False)
    mt = nc.declare_dram_parameter("mt", [H, S], dt.float32r, isOutput=False)
    mtb = nc.declare_dram_parameter("mtb", [NJB, P, H], BF, isOutput=False)
    wq = nc.declare_dram_parameter("wq", [H, H], dt.float32r, isOutput=False)
    wk = nc.declare_dram_parameter("wk", [H, H], dt.float32r, isOutput=False)
    wv = nc.declare_dram_parameter("wv", [H, H], BF, isOutput=False)
    wo = nc.declare_dram_parameter("wo", [H, H], BFO, isOutput=False)
    bias = nc.declare_dram_parameter("bias", [S, S], dt.float32, isOutput=False)
    bqp = bkp = bvp = bop = None
    if nonzero_bq:
        bqp = nc.declare_dram_parameter("bq", [P, HC], dt.float32, isOutput=False)
    if nonzero_bk:
        bkp = nc.declare_dram_parameter("bk", [P, HC], dt.float32, isOutput=False)
    if nonzero_bv:
        bvp = nc.declare_dram_parameter("bv", [P, H], dt.float32, isOutput=False)
    if nonzero_bo:
        bop = nc.declare_dram_parameter("bo", [P, H], dt.float32, isOutput=False)
    attn_out = nc.declare_dram_parameter("attn", [S, S], dt.float32, isOutput=True)
    ret_out = nc.declare_dram_parameter("ret", [S, H], dt.float32, isOutput=True)

    # qT scratch, blocked per 128-row block: qts[bi, p, hc*128+i]
    qts = nc.dram_tensor("qts", [NJB, P, H], dt.float32r)

    inv_sqrt_h = 1.0 / float(np.sqrt(H))

    with tile.TileContext(nc) as tc:
        with (
            tc.tile_pool(name="const", bufs=1) as cpool,
            tc.tile_pool(name="kt", bufs=HC) as kt_pool,
            tc.tile_pool(name="psum", bufs=8, space="PSUM") as pp,
        ):
            ident = cpool.tile([P, P], dt.float32)
            make_identity(nc, ident[:])
            bq_sb = bk_sb = bv_sb = bo_sb = None
            if bqp is not None:
                bq_sb = cpool.tile([P, HC], dt.float32)
                nc.sync.dma_start(bq_sb[:], bqp[:])
            if bkp is not None:
                bk_sb = cpool.tile([P, HC], dt.float32)
                nc.sync.dma_start(bk_sb[:], bkp[:])
            if bvp is not None:
                bv_sb = cpool.tile([P, H], dt.float32)
                nc.sync.dma_start(bv_sb[:], bvp[:])
            if bop is not None:
                bo_sb = cpool.tile([P, H], dt.float32)
                nc.sync.dma_start(bo_sb[:], bop[:])

            kt_sb = [kt_pool.tile([P, S], dt.float32r, tag="kt", name="kt")
                     for _ in range(HC)]

            # ---- Phase 0: qT and kT, interleaved per 512-column block ----
            with (
                tc.tile_pool(name="w", bufs=16) as wpool,
                tc.tile_pool(name="xm", bufs=32) as xmpool,
                tc.tile_pool(name="qo", bufs=4) as qopool,
            ):
                wq_t, wk_t, xt0_t, mt0_t = [], [], [], []
                for c in range(DC):
                    # interleave first-block state loads with weight loads so
                    # the first matmul group starts early
                    t0 = xmpool.tile([P, 512], dt.float32r, tag="xm", name="xm")
                    nc.sync.dma_start(t0[:], xt[c * P:(c + 1) * P, 0:512])
                    xt0_t.append(t0)
                    t = wpool.tile([P, H], dt.float32r, tag="w", name="w")
                    nc.sync.dma_start(t[:], wq[c * P:(c + 1) * P, :])
                    wq_t.append(t)

                xt_blk = {0: xt0_t}
                mt_blk = {0: mt0_t}

                def load_xm(which, ic):
                    blk = []
                    base = xt if which == "x" else mt
                    for c in range(DC):
                        t = xmpool.tile([P, 512], dt.float32r, tag="xm",
                                        name="xm")
                        nc.sync.dma_start(
                            t[:], base[c * P:(c + 1) * P,
                                       ic * 512:(ic + 1) * 512])
                        blk.append(t)
                    return blk

                def emit_qT(ic):
                    xt_t = xt_blk.pop(ic)
                    for hb in range(HC):
                        ps = pp.tile([P, 512], dt.float32, tag="ps", name="ps")
                        for c in range(DC):
                            nc.tensor.matmul(
                                ps[:],
                                wq_t[c][:, hb * P:(hb + 1) * P],
                                xt_t[c][:],
                                start=(c == 0),
                                stop=(c == DC - 1),
                            )
                        qsb = qopool.tile([P, 512], dt.float32r, tag="qo",
                                          name="qo")
                        nc.vector.tensor_scalar(
                            qsb[:],
                            ps[:],
                            bq_sb[:, hb:hb + 1] if bq_sb is not None else 0.0,
                            inv_sqrt_h,
                            op0=OP.add,
                            op1=OP.mult,
                        )
                        for r in range(4):
                            bi = ic * 4 + r
                            nc.sync.dma_start(
                                qts[bi, :, hb * P:(hb + 1) * P],
                                qsb[:, r * P:(r + 1) * P],
                            )

                def emit_kT(ic):
                    mt_t = mt_blk.pop(ic)
                    for hb in range(HC):
                        ps = pp.tile([P, 512], dt.float32, tag="ps", name="ps")
                        for c in range(DC):
                            nc.tensor.matmul(
                                ps[:],
                                wk_t[c][:, hb * P:(hb + 1) * P],
                                mt_t[c][:],
                                start=(c == 0),
                                stop=(c == DC - 1),
                            )
                        nc.vector.tensor_scalar(
                            kt_sb[hb][:, ic * 512:(ic + 1) * 512],
                            ps[:],
                            bk_sb[:, hb:hb + 1] if bk_sb is not None else 0.0,
                            None,
                            op0=OP.add,
                        )

                # schedule: qT one block ahead of kT; loads ordered to
                # keep the DMA queue just ahead of the PE
                xt_blk[1] = load_xm("x", 1)
                emit_qT(0)
                for c in range(DC):
                    t0 = xmpool.tile([P, 512], dt.float32r, tag="xm", name="xm")
                    nc.sync.dma_start(t0[:], mt[c * P:(c + 1) * P, 0:512])
                    mt0_t.append(t0)
                    t = wpool.tile([P, H], dt.float32r, tag="w", name="w")
                    nc.sync.dma_start(t[:], wk[c * P:(c + 1) * P, :])
                    wk_t.append(t)
                xt_blk[2] = load_xm("x", 2)
                emit_qT(1)
                mt_blk[1] = load_xm("m", 1)
                emit_kT(0)
                xt_blk[3] = load_xm("x", 3)
                emit_qT(2)
                mt_blk[2] = load_xm("m", 2)
                mt_blk[3] = load_xm("m", 3)
                emit_kT(1)
                emit_qT(3)
                emit_kT(2)
                emit_kT(3)

            # ---- Phase 1: v projection (as PE filler) + attention ----
            with (
                tc.tile_pool(name="v", bufs=NJB) as v_pool,
                tc.tile_pool(name="wv", bufs=8) as wvpool,
                tc.tile_pool(name="mtb", bufs=4) as mtbpool,
                tc.tile_pool(name="at", bufs=16) as atpool,
                tc.tile_pool(name="qt", bufs=4) as qtpool,
                tc.tile_pool(name="s", bufs=4) as spool,
                tc.tile_pool(name="rt", bufs=8) as rtpool,
                tc.tile_pool(name="wo", bufs=8) as wopool,
                tc.tile_pool(name="og", bufs=2) as ogpool,
                tc.tile_pool(name="st", bufs=3) as stpool,
            ):
                v_sb = [v_pool.tile([P, H], BF, tag="v", name="v")
                        for _ in range(NJB)]
                tmp0pool = None
                if bv_sb is not None or bo_sb is not None:
                    tmp0pool = tc.tile_pool(name="tmp0", bufs=2).__enter__()
                wv_t = []

                def load_wv():
                    for c in range(DC):
                        t = wvpool.tile([P, H], BF, tag="wv",
                                        name="wv")
                        nc.sync.dma_start(t[:], wv[c * P:(c + 1) * P, :])
                        wv_t.append(t)

                def emit_v(jb):
                    # v[jb] = gelu(MT.T @ Wv + bv) -> resident (bf16)
                    mtb_t = mtbpool.tile([P, H], BF, tag="mtb",
                                         name="mtb")
                    nc.sync.dma_start(mtb_t[:], mtb[jb])
                    for hcol in range(H // 512):
                        ps = pp.tile([P, 512], dt.float32, tag="ps", name="ps")
                        for c in range(DC):
                            nc.tensor.matmul(
                                ps[:],
                                mtb_t[:, c * P:(c + 1) * P],
                                wv_t[c][:, hcol * 512:(hcol + 1) * 512],
                                start=(c == 0),
                                stop=(c == DC - 1),
                            )
                        dst = v_sb[jb][:, hcol * 512:(hcol + 1) * 512]
                        if bv_sb is not None:
                            tmp = tmp0pool.tile([P, 512], dt.float32,
                                                tag="tmp0", name="tmp0")
                            nc.vector.tensor_tensor(
                                tmp[:], ps[:],
                                bv_sb[:, hcol * 512:(hcol + 1) * 512], OP.add,
                            )
                            nc.scalar.activation(dst, tmp[:], GELU)
                        else:
                            nc.scalar.activation(dst, ps[:], GELU)

                qt_tiles = {}

                def ensure_qt(bi):
                    if bi not in qt_tiles and bi < NJB:
                        t = qtpool.tile([P, H], dt.float32r, tag="qt",
                                        name="qt")
                        nc.sync.dma_start(t[:], qts[bi])
                        qt_tiles[bi] = t

                def emit_scores(bi, njt, Jb, s_tiles):
                    ensure_qt(bi)
                    ensure_qt(bi + 1)
                    ensure_qt(bi + 2)
                    ensure_qt(bi + 3)
                    qt_t = qt_tiles.pop(bi)
                    Sx = spool.tile([P, S], dt.float32, tag="s", name="s")
                    s_tiles[bi] = Sx
                    nc.sync.dma_start(Sx[:, :Jb], bias[bi * P:(bi + 1) * P, :Jb])
                    mx4 = stpool.tile([P, 4], dt.float32, tag="mx4", name="mx4")
                    for jt in range(njt):
                        ps = pp.tile([P, 512], dt.float32, tag="ps", name="ps")
                        for hcn in range(HC):
                            nc.tensor.matmul(
                                ps[:],
                                qt_t[:, hcn * P:(hcn + 1) * P],
                                kt_sb[hcn][:, jt * 512:(jt + 1) * 512],
                                start=(hcn == 0),
                                stop=(hcn == HC - 1),
                            )
                        nc.vector.tensor_tensor(
                            Sx[:, jt * 512:(jt + 1) * 512],
                            ps[:],
                            Sx[:, jt * 512:(jt + 1) * 512],
                            OP.add,
                        )
                        nc.vector.reduce_max(
                            mx4[:, jt:jt + 1],
                            Sx[:, jt * 512:(jt + 1) * 512],
                            axis=AX.X,
                        )
                    nmx = stpool.tile([P, 1], dt.float32, tag="nmx", name="nmx")
                    nc.vector.reduce_max(nmx[:], mx4[:, :njt], axis=AX.X,
                                         negate=True)
                    sume = stpool.tile([P, 1], dt.float32, tag="sume",
                                       name="sume")
                    nc.scalar.activation(
                        Sx[:, :Jb], Sx[:, :Jb], AF.Exp,
                        bias=nmx[:], accum_out=sume[:],
                    )
                    rcp = stpool.tile([P, 1], dt.float32, tag="rcp", name="rcp")
                    nc.vector.reciprocal(rcp[:], sume[:])
                    flag = stpool.tile([P, 1], dt.float32, tag="flag",
                                       name="flag")
                    nc.vector.tensor_scalar(flag[:], nmx[:], 1.0e29, None,
                                            op0=OP.is_lt)
                    rcpz = stpool.tile([P, 1], dt.float32, tag="rcpz",
                                       name="rcpz")
                    nc.vector.tensor_tensor(rcpz[:], rcp[:], flag[:], OP.mult)
                    nc.vector.tensor_scalar(Sx[:, :Jb], Sx[:, :Jb], rcpz[:],
                                            None, op0=OP.mult)
                    nc.sync.dma_start(attn_out[bi * P:(bi + 1) * P, :Jb],
                                      Sx[:, :Jb])

                def emit_transposes(bi, njt, s_tiles, at):
                    Sx = s_tiles[bi]
                    col = (bi % 4) * P
                    for g in range(njt):
                        pst = pp.tile([P, 512], dt.float32, tag="ps", name="ps")
                        for k in range(4):
                            nc.tensor.transpose(
                                pst[:, k * P:(k + 1) * P],
                                Sx[:, (4 * g + k) * P:(4 * g + k + 1) * P],
                                ident[:],
                            )
                        for k in range(4):
                            nc.any.tensor_copy(
                                out=at[4 * g + k][:, col:col + P],
                                in_=pst[:, k * P:(k + 1) * P],
                            )

                def emit_outproj(b4, rt):
                    for gcol in range(H // 512):
                        ps_o = [pp.tile([P, 512], dt.float32, tag="ps",
                                        name="ps") for _ in range(4)]
                        for hb in range(HC):
                            wo_t = wopool.tile([P, 512], BFO, tag="wo",
                                               name="wo")
                            nc.sync.dma_start(
                                wo_t[:],
                                wo[hb * P:(hb + 1) * P,
                                   gcol * 512:(gcol + 1) * 512],
                            )
                            for isub in range(4):
                                nc.tensor.matmul(
                                    ps_o[isub][:],
                                    rt[hb][:, isub * P:(isub + 1) * P],
                                    wo_t[:],
                                    start=(hb == 0),
                                    stop=(hb == HC - 1),
                                )
                        for isub in range(4):
                            og = ogpool.tile([P, 512], dt.float32, tag="og",
                                             name="og")
                            if bo_sb is not None:
                                nc.vector.tensor_tensor(
                                    og[:], ps_o[isub][:],
                                    bo_sb[:, gcol * 512:(gcol + 1) * 512],
                                    OP.add,
                                )
                                nc.scalar.activation(og[:], og[:], GELU)
                            else:
                                nc.scalar.activation(og[:], ps_o[isub][:], GELU)
                            row = (4 * b4 + isub) * P
                            nc.sync.dma_start(
                                ret_out[row:row + P,
                                        gcol * 512:(gcol + 1) * 512],
                                og[:],
                            )

                pending_outproj = None  # deferred to fill PE gaps
                for b4 in range(NB if PHASE >= 1 else 0):
                    Jb = 512 * (b4 + 1)
                    njt = b4 + 1
                    njc = 4 * (b4 + 1)
                    at = [atpool.tile([P, 512], BF, tag="at",
                                      name="at") for _ in range(njc)]
                    s_tiles = {}

                    prev = None
                    for bi in range(4 * b4, 4 * b4 + 4):
                        emit_scores(bi, njt, Jb, s_tiles)
                        if b4 == 0 and bi == 0:
                            load_wv()
                        if prev is not None and PHASE >= 2:
                            emit_transposes(prev, njt, s_tiles, at)
                        prev = bi
                    # fill the softmax tail with the previous block's
                    # out-projection and this block's v tiles
                    if pending_outproj is not None:
                        emit_outproj(*pending_outproj)
                        pending_outproj = None
                    for jb in range(4 * b4, 4 * b4 + 4):
                        emit_v(jb)
                    if PHASE >= 2:
                        emit_transposes(prev, njt, s_tiles, at)
                    if PHASE < 3:
                        continue

                    # ret_preT[h, i] = sum_j v[j, h] * attnT[j, i]
                    rt = [rtpool.tile([P, 512], BFO, tag="rt",
                                      name="rt") for _ in range(HC)]
                    for hb in range(HC):
                        ps = pp.tile([P, 512], dt.float32, tag="ps", name="ps")
                        for jc in range(njc):
                            nc.tensor.matmul(
                                ps[:],
                                v_sb[jc][:, hb * P:(hb + 1) * P],
                                at[jc][:],
                                start=(jc == 0),
                                stop=(jc == njc - 1),
                            )
                        nc.any.tensor_copy(out=rt[hb][:], in_=ps[:])
                    if NODEFER:
                        emit_outproj(b4, rt)
                    else:
                        pending_outproj = (b4, rt)

                if pending_outproj is not None:
                    emit_outproj(*pending_outproj)
                if tmp0pool is not None:
                    tmp0pool.__exit__(None, None, None)

    nc.finalize()
    return nc


def _make_runner(nc):
    """Build a cached multi-core PJRT runner for a finalized Bass program.

    Mirrors concourse.bass2jax.run_bass_via_pjrt's multi-core path, but keeps
    the jitted callable so repeated invocations don't re-trace/re-compile.
    """
    import jax
    import concourse.mybir as mybir
    from jax.experimental.shard_map import shard_map
    from jax.sharding import Mesh, PartitionSpec
    from concourse import bass2jax as b2j

    b2j.install_neuronx_cc_hook()

    partition_name = (
        nc.partition_id_tensor.name if nc.partition_id_tensor else None
    )
    in_names, out_names, out_avals, zero_shapes = [], [], [], []
    for alloc in nc.m.functions[0].allocations:
        if not isinstance(alloc, mybir.MemoryLocationSet):
            continue
        name = alloc.memorylocations[0].name
        if alloc.kind == "ExternalInput":
            if name != partition_name:
                in_names.append(name)
        elif alloc.kind == "ExternalOutput":
            shape = tuple(alloc.tensor_shape)
            dtype = mybir.dt.np(alloc.dtype)
            out_names.append(name)
            out_avals.append(jax.core.ShapedArray(shape, dtype))
            zero_shapes.append((shape, dtype))
    n_params = len(in_names)
    n_outs = len(out_avals)
    all_in_names = list(in_names) + list(out_names)
    if partition_name is not None:
        all_in_names.append(partition_name)
    donate = tuple(range(n_params, n_params + n_outs))

    def _body(*args):
        operands = list(args)
        if partition_name is not None:
            operands.append(b2j.partition_id_tensor())
        outs = b2j._bass_exec_p.bind(
            *operands,
            out_avals=tuple(out_avals),
            in_names=tuple(all_in_names),
            out_names=tuple(out_names),
            lowering_input_output_aliases=(),
            sim_require_finite=True,
            sim_require_nnan=True,
            nc=nc,
        )
        return tuple(outs)

    devices = jax.devices()[:N_CORES]
    mesh = Mesh(np.asarray(devices), ("core",))
    in_specs = (PartitionSpec("core"),) * (n_params + n_outs)
    out_specs = (PartitionSpec("core"),) * n_outs
    sharded = jax.jit(
        shard_map(
            _body, mesh=mesh, in_specs=in_specs, out_specs=out_specs,
            check_rep=False,
        ),
        donate_argnums=donate,
        keep_unused=True,
    )

    def make_zeros():
        return [
            np.zeros((N_CORES * shp[0], *shp[1:]), dtp)
            for shp, dtp in zero_shapes
        ]

    def concat_inputs(in_maps):
        return [
            np.concatenate([np.asarray(in_maps[c][k]) for c in range(N_CORES)],
                           axis=0)
            for k in in_names
        ]

    def run(in_maps):
        out_arrs = sharded(*concat_inputs(in_maps), *make_zeros())
        return [
            {
                k: np.asarray(out_arrs[i]).reshape(
                    N_CORES, *out_avals[i].shape
                )[c]
                for i, k in enumerate(out_names)
            }
            for c in range(N_CORES)
        ]

    run.sharded = sharded
    run.concat_inputs = concat_inputs
    run.make_zeros = make_zeros
    run.out_names = out_names
    run.out_avals = out_avals
    return run


def _prepare_in_maps(query_state, memory_state, time_seq, Wq, bq, Wk, bk,
                     Wv, bv, Wo, bo, order_scale, time_scale, padding_mask):
    import ml_dtypes

    bf = np.float32 if os.environ.get("K_F32RET") else ml_dtypes.bfloat16
    bfo = np.float32 if os.environ.get("K_F32OP") else ml_dtypes.bfloat16
    q = np.asarray(query_state, dtype=np.float32)
    m = np.asarray(memory_state, dtype=np.float32)
    t = np.asarray(time_seq, dtype=np.float32)
    pad = np.asarray(padding_mask).astype(bool)
    Wq = np.ascontiguousarray(np.asarray(Wq, dtype=np.float32))
    Wk = np.ascontiguousarray(np.asarray(Wk, dtype=np.float32))
    Wv16 = np.ascontiguousarray(
        np.asarray(Wv, dtype=np.float32).astype(bf)
    )
    Wo16 = np.ascontiguousarray(
        np.asarray(Wo, dtype=np.float32).astype(bfo)
    )
    bq = np.asarray(bq, dtype=np.float32)
    bk = np.asarray(bk, dtype=np.float32)
    bv = np.asarray(bv, dtype=np.float32)
    bo = np.asarray(bo, dtype=np.float32)

    def softplus(x):
        x = float(np.asarray(x))
        return float(np.log1p(np.exp(-abs(x))) + max(x, 0.0))

    co = np.float32(-softplus(order_scale))
    ct = np.float32(-softplus(time_scale))

    idx = np.arange(S, dtype=np.float32)
    od = np.maximum(idx[:, None] - idx[None, :], 0.0)
    ob = co * np.log1p(od)  # [S,S] f32
    tri = np.triu(np.ones((S, S), dtype=bool))  # strictly-causal invalid

    flags = (bool(np.any(bq)), bool(np.any(bk)),
             bool(np.any(bv)), bool(np.any(bo)))

    in_maps = []
    for b in range(B):
        tg = np.maximum(t[b][:, None] - t[b][None, :], 0.0)
        bias_b = ob + ct * np.log1p(tg)
        invalid = tri | pad[b][None, :]
        bias_b[invalid] = NEG
        # blocked MT copy: mtb[jb, p, dc*128 + j] = M[jb*128+j, dc*128+p]
        mtb = np.ascontiguousarray(
            m[b].astype(bf)
            .reshape(NJB, P, DC, P).transpose(0, 3, 2, 1)
            .reshape(NJB, P, H)
        )
        im = {
            "xt": np.ascontiguousarray(q[b].T),
            "mt": np.ascontiguousarray(m[b].T),
            "mtb": mtb,
            "wq": Wq, "wk": Wk, "wv": Wv16, "wo": Wo16,
            "bias": np.ascontiguousarray(bias_b, dtype=np.float32),
        }
        if flags[0]:
            im["bq"] = np.ascontiguousarray(bq.reshape(HC, P).T)
        if flags[1]:
            im["bk"] = np.ascontiguousarray(bk.reshape(HC, P).T)
        if flags[2]:
            im["bv"] = np.ascontiguousarray(np.broadcast_to(bv, (P, H)))
        if flags[3]:
            im["bo"] = np.ascontiguousarray(np.broadcast_to(bo, (P, H)))
        in_maps.append(im)
    return in_maps, flags


def _get_runner(flags):
    global _RUNNER
    if _RUNNER is None or _RUNNER[1] != flags:
        nc = _build_nc(*flags)
        _RUNNER = (_make_runner(nc), flags)
    return _RUNNER[0]


def kernel(**inputs):
    in_maps, flags = _prepare_in_maps(**inputs)
    run = _get_runner(flags)
    res = run(in_maps)
    retrieved = np.stack([res[c]["ret"] for c in range(N_CORES)])
    attn = np.stack([res[c]["attn"] for c in range(N_CORES)])
    return retrieved, attn


# revision 23
# speedup vs baseline: 36.4033x; 1.3413x over previous
"""AdaptiveStateMixer kernel for 8 Trainium2 NeuronCores.

Strategy:
  - Data parallel over batch: B=8 batches -> one batch per core, no collectives.
  - Host precomputes: transposed states XT/MT ([H,S]), a block-transposed copy
    of MT (for large contiguous DMA loads in the value projection), and a
    combined additive bias+mask matrix per batch ([S,S]) holding
    order_bias + time_bias, with -1e30 at invalid (causal/padded) slots.
  - On chip (per core): qT = (Wq.T@XT + bq)/sqrt(H) (roundtripped through a
    blocked DRAM scratch), kT = Wk.T@MT + bk (SBUF-resident),
    v = gelu(MT.T@Wv + bv); scores = qT.T@kT + bias; softmax along the free
    dim (rows with no valid key get zeroed via a row-max flag); attn written
    to DRAM; attn transposed via PE transpose; ret = gelu((attn @ v) @ Wo+bo).
  - The q/k/score path runs in float32r (full-rate fp32 matmul, ~1e-4 err);
    the post-softmax value path (v, attnT, ret_pre, Wo) runs in bf16.
  - Strictly causal: score/attn-v tiles entirely above the diagonal are
    skipped; the attn output relies on the runtime's pre-zeroed output
    buffers for the never-written upper triangle.
"""

import os
import sys

sys.path.insert(0, "/opt/trn_rl_repo")

import numpy as np

B, S, H, P = 8, 2048, 1024, 128
NEG = np.float32(-1e30)
HC = H // P     # 8 h-chunks
DC = H // P     # 8 d-chunks
NB = S // 512   # 4 row blocks of 512
NJB = S // P    # 16 column blocks of 128
N_CORES = 8

_RUNNER = None  # (run_fn, build_key)


def _build_nc(nonzero_bq, nonzero_bk, nonzero_bv, nonzero_bo):
    import concourse.mybir as mybir
    import concourse.tile as tile
    from concourse import bacc
    from concourse.masks import make_identity

    dt = mybir.dt
    AF = mybir.ActivationFunctionType
    GELU = AF.Identity if os.environ.get("K_NOGELU") else AF.Gelu
    OP = mybir.AluOpType
    AX = mybir.AxisListType
    PHASE = int(os.environ.get("K_PHASE", "9"))
    NODEFER = not bool(os.environ.get("K_DEFER"))
    F32OP = bool(os.environ.get("K_F32OP"))
    F32RET = bool(os.environ.get("K_F32RET"))
    BF = dt.float32r if F32RET else dt.bfloat16
    BFO = dt.float32r if F32OP else dt.bfloat16

    nc = bacc.Bacc(None, target_bir_lowering=False)

    xt = nc.declare_dram_parameter("xt", [H, S], dt.float32r, isOutput=False)
    mt = nc.declare_dram_parameter("mt", [H, S], dt.float32r, isOutput=False)
    mtb = nc.declare_dram_parameter("mtb", [NJB, P, H], BF, isOutput=False)
    wq = nc.declare_dram_parameter("wq", [H, H], dt.float32r, isOutput=False)
    wk = nc.declare_dram_parameter("wk", [H, H], dt.float32r, isOutput=False)
    wv = nc.declare_dram_parameter("wv", [H, H], BF, isOutput=False)
    wo = nc.declare_dram_parameter("wo", [H, H], BFO, isOutput=False)
    bias = nc.declare_dram_parameter("bias", [S, S], dt.float32, isOutput=False)
    bqp = bkp = bvp = bop = None
    if nonzero_bq:
        bqp = nc.declare_dram_parameter("bq", [P, HC], dt.float32, isOutput=False)
    if nonzero_bk:
        bkp = nc.declare_dram_parameter("bk", [P, HC], dt.float32, isOutput=False)
    if nonzero_bv:
        bvp = nc.declare_dram_parameter("bv", [P, H], dt.float32, isOutput=False)
    if nonzero_bo:
        bop = nc.declare_dram_parameter("bo", [P, H], dt.float32, isOutput=False)
    attn_out = nc.declare_dram_parameter("attn", [S, S], dt.float32, isOutput=True)
    ret_out = nc.declare_dram_parameter("ret", [S, H], dt.float32, isOutput=True)

    # qT scratch, blocked per 128-row block: qts[bi, p, hc*128+i]
    qts = nc.dram_tensor("qts", [NJB, P, H], dt.float32r)

    inv_sqrt_h = 1.0 / float(np.sqrt(H))

    with tile.TileContext(nc) as tc:
        with (
            tc.tile_pool(name="const", bufs=1) as cpool,
            tc.tile_pool(name="kt", bufs=HC) as kt_pool,
            tc.tile_pool(name="psum", bufs=8, space="PSUM") as pp,
        ):
            ident = cpool.tile([P, P], dt.float32)
            make_identity(nc, ident[:])
            bq_sb = bk_sb = bv_sb = bo_sb = None
            if bqp is not None:
                bq_sb = cpool.tile([P, HC], dt.float32)
                nc.sync.dma_start(bq_sb[:], bqp[:])
            if bkp is not None:
                bk_sb = cpool.tile([P, HC], dt.float32)
                nc.sync.dma_start(bk_sb[:], bkp[:])
            if bvp is not None:
                bv_sb = cpool.tile([P, H], dt.float32)
                nc.sync.dma_start(bv_sb[:], bvp[:])
            if bop is not None:
                bo_sb = cpool.tile([P, H], dt.float32)
                nc.sync.dma_start(bo_sb[:], bop[:])

            kt_sb = [kt_pool.tile([P, S], dt.float32r, tag="kt", name="kt")
                     for _ in range(HC)]

            # ---- Phase 0: qT and kT, interleaved per 512-column block ----
            with (
                tc.tile_pool(name="w", bufs=16) as wpool,
                tc.tile_pool(name="xm", bufs=32) as xmpool,
                tc.tile_pool(name="qo", bufs=6) as qopool,
            ):
                wq_t, wk_t, xt0_t, mt0_t = [], [], [], []
                for c in range(DC):
                    # interleave first-block state loads with weight loads so
                    # the first matmul group starts early
                    t0 = xmpool.tile([P, 512], dt.float32r, tag="xm", name="xm")
                    nc.sync.dma_start(t0[:], xt[c * P:(c + 1) * P, 0:512])
                    xt0_t.append(t0)
                    t = wpool.tile([P, H], dt.float32r, tag="w", name="w")
                    nc.sync.dma_start(t[:], wq[c * P:(c + 1) * P, :])
                    wq_t.append(t)

                xt_blk = {0: xt0_t}
                mt_blk = {0: mt0_t}

                def load_xm(which, ic):
                    blk = []
                    base = xt if which == "x" else mt
                    for c in range(DC):
                        t = xmpool.tile([P, 512], dt.float32r, tag="xm",
                                        name="xm")
                        nc.sync.dma_start(
                            t[:], base[c * P:(c + 1) * P,
                                       ic * 512:(ic + 1) * 512])
                        blk.append(t)
                    return blk

                def emit_qT(ic):
                    xt_t = xt_blk.pop(ic)
                    for hb in range(HC):
                        ps = pp.tile([P, 512], dt.float32, tag="ps", name="ps")
                        for c in range(DC):
                            nc.tensor.matmul(
                                ps[:],
                                wq_t[c][:, hb * P:(hb + 1) * P],
                                xt_t[c][:],
                                start=(c == 0),
                                stop=(c == DC - 1),
                            )
                        qsb = qopool.tile([P, 512], dt.float32r, tag="qo",
                                          name="qo")
                        nc.vector.tensor_scalar(
                            qsb[:],
                            ps[:],
                            bq_sb[:, hb:hb + 1] if bq_sb is not None else 0.0,
                            inv_sqrt_h,
                            op0=OP.add,
                            op1=OP.mult,
                        )
                        for r in range(4):
                            bi = ic * 4 + r
                            nc.sync.dma_start(
                                qts[bi, :, hb * P:(hb + 1) * P],
                                qsb[:, r * P:(r + 1) * P],
                            )

                def emit_kT(ic):
                    mt_t = mt_blk.pop(ic)
                    for hb in range(HC):
                        ps = pp.tile([P, 512], dt.float32, tag="ps", name="ps")
                        for c in range(DC):
                            nc.tensor.matmul(
                                ps[:],
                                wk_t[c][:, hb * P:(hb + 1) * P],
                                mt_t[c][:],
                                start=(c == 0),
                                stop=(c == DC - 1),
                            )
                        nc.vector.tensor_scalar(
                            kt_sb[hb][:, ic * 512:(ic + 1) * 512],
                            ps[:],
                            bk_sb[:, hb:hb + 1] if bk_sb is not None else 0.0,
                            None,
                            op0=OP.add,
                        )

                # schedule: qT one block ahead of kT; loads ordered to
                # keep the DMA queue just ahead of the PE
                xt_blk[1] = load_xm("x", 1)
                emit_qT(0)
                for c in range(DC):
                    t0 = xmpool.tile([P, 512], dt.float32r, tag="xm", name="xm")
                    nc.sync.dma_start(t0[:], mt[c * P:(c + 1) * P, 0:512])
                    mt0_t.append(t0)
                    t = wpool.tile([P, H], dt.float32r, tag="w", name="w")
                    nc.sync.dma_start(t[:], wk[c * P:(c + 1) * P, :])
                    wk_t.append(t)
                xt_blk[2] = load_xm("x", 2)
                emit_qT(1)
                mt_blk[1] = load_xm("m", 1)
                emit_kT(0)
                xt_blk[3] = load_xm("x", 3)
                emit_qT(2)
                mt_blk[2] = load_xm("m", 2)
                mt_blk[3] = load_xm("m", 3)
                emit_kT(1)
                emit_qT(3)
                emit_kT(2)
                emit_kT(3)

            # ---- Phase 1: v projection (as PE filler) + attention ----
            with (
                tc.tile_pool(name="v", bufs=NJB) as v_pool,
                tc.tile_pool(name="wv", bufs=8) as wvpool,
                tc.tile_pool(name="mtb", bufs=4) as mtbpool,
                tc.tile_pool(name="at", bufs=16) as atpool,
                tc.tile_pool(name="qt", bufs=4) as qtpool,
                tc.tile_pool(name="s", bufs=4) as spool,
                tc.tile_pool(name="rt", bufs=8) as rtpool,
                tc.tile_pool(name="wo", bufs=8) as wopool,
                tc.tile_pool(name="og", bufs=3) as ogpool,
                tc.tile_pool(name="st", bufs=3) as stpool,
            ):
                v_sb = [v_pool.tile([P, H], BF, tag="v", name="v")
                        for _ in range(NJB)]
                tmp0pool = None
                if bv_sb is not None or bo_sb is not None:
                    tmp0pool = tc.tile_pool(name="tmp0", bufs=2).__enter__()
                wv_t = []

                def load_wv():
                    for c in range(DC):
                        t = wvpool.tile([P, H], BF, tag="wv",
                                        name="wv")
                        nc.sync.dma_start(t[:], wv[c * P:(c + 1) * P, :])
                        wv_t.append(t)

                def emit_v(jb):
                    # v[jb] = gelu(MT.T @ Wv + bv) -> resident (bf16)
                    mtb_t = mtbpool.tile([P, H], BF, tag="mtb",
                                         name="mtb")
                    nc.sync.dma_start(mtb_t[:], mtb[jb])
                    for hcol in range(H // 512):
                        ps = pp.tile([P, 512], dt.float32, tag="ps", name="ps")
                        for c in range(DC):
                            nc.tensor.matmul(
                                ps[:],
                                mtb_t[:, c * P:(c + 1) * P],
                                wv_t[c][:, hcol * 512:(hcol + 1) * 512],
                                start=(c == 0),
                                stop=(c == DC - 1),
                            )
                        dst = v_sb[jb][:, hcol * 512:(hcol + 1) * 512]
                        if bv_sb is not None:
                            tmp = tmp0pool.tile([P, 512], dt.float32,
                                                tag="tmp0", name="tmp0")
                            nc.vector.tensor_tensor(
                                tmp[:], ps[:],
                                bv_sb[:, hcol * 512:(hcol + 1) * 512], OP.add,
                            )
                            nc.scalar.activation(dst, tmp[:], GELU)
                        else:
                            nc.scalar.activation(dst, ps[:], GELU)

                qt_tiles = {}
                sb_tiles = {}

                def ensure_qt(bi):
                    if bi not in qt_tiles and bi < NJB:
                        t = qtpool.tile([P, H], dt.float32r, tag="qt",
                                        name="qt")
                        nc.sync.dma_start(t[:], qts[bi])
                        qt_tiles[bi] = t

                def ensure_sbias(bi):
                    # preload the bias rows for row-block bi into its S tile
                    if bi not in sb_tiles and bi < NJB:
                        Jbi = 512 * (bi // 4 + 1)
                        Sx = spool.tile([P, S], dt.float32, tag="s", name="s")
                        nc.sync.dma_start(
                            Sx[:, :Jbi], bias[bi * P:(bi + 1) * P, :Jbi]
                        )
                        sb_tiles[bi] = Sx

                def emit_scores(bi, njt, Jb, s_tiles):
                    ensure_qt(bi)
                    ensure_qt(bi + 1)
                    ensure_qt(bi + 2)
                    ensure_qt(bi + 3)
                    ensure_sbias(bi)
                    ensure_sbias(bi + 1)
                    qt_t = qt_tiles.pop(bi)
                    Sx = sb_tiles.pop(bi)
                    s_tiles[bi] = Sx
                    mx4 = stpool.tile([P, 4], dt.float32, tag="mx4", name="mx4")
                    for jt in range(njt):
                        ps = pp.tile([P, 512], dt.float32, tag="ps", name="ps")
                        for hcn in range(HC):
                            nc.tensor.matmul(
                                ps[:],
                                qt_t[:, hcn * P:(hcn + 1) * P],
                                kt_sb[hcn][:, jt * 512:(jt + 1) * 512],
                                start=(hcn == 0),
                                stop=(hcn == HC - 1),
                            )
                        nc.vector.tensor_tensor(
                            Sx[:, jt * 512:(jt + 1) * 512],
                            ps[:],
                            Sx[:, jt * 512:(jt + 1) * 512],
                            OP.add,
                        )
                        nc.vector.reduce_max(
                            mx4[:, jt:jt + 1],
                            Sx[:, jt * 512:(jt + 1) * 512],
                            axis=AX.X,
                        )
                    nmx = stpool.tile([P, 1], dt.float32, tag="nmx", name="nmx")
                    nc.vector.reduce_max(nmx[:], mx4[:, :njt], axis=AX.X,
                                         negate=True)
                    sume = stpool.tile([P, 1], dt.float32, tag="sume",
                                       name="sume")
                    nc.scalar.activation(
                        Sx[:, :Jb], Sx[:, :Jb], AF.Exp,
                        bias=nmx[:], accum_out=sume[:],
                    )
                    rcp = stpool.tile([P, 1], dt.float32, tag="rcp", name="rcp")
                    nc.vector.reciprocal(rcp[:], sume[:])
                    flag = stpool.tile([P, 1], dt.float32, tag="flag",
                                       name="flag")
                    nc.vector.tensor_scalar(flag[:], nmx[:], 1.0e29, None,
                                            op0=OP.is_lt)
                    rcpz = stpool.tile([P, 1], dt.float32, tag="rcpz",
                                       name="rcpz")
                    nc.vector.tensor_tensor(rcpz[:], rcp[:], flag[:], OP.mult)
                    nc.vector.tensor_scalar(Sx[:, :Jb], Sx[:, :Jb], rcpz[:],
                                            None, op0=OP.mult)
                    nc.sync.dma_start(attn_out[bi * P:(bi + 1) * P, :Jb],
                                      Sx[:, :Jb])

                def emit_transposes(bi, njt, s_tiles, at):
                    Sx = s_tiles[bi]
                    col = (bi % 4) * P
                    for g in range(njt):
                        pst = pp.tile([P, 512], dt.float32, tag="ps", name="ps")
                        for k in range(4):
                            nc.tensor.transpose(
                                pst[:, k * P:(k + 1) * P],
                                Sx[:, (4 * g + k) * P:(4 * g + k + 1) * P],
                                ident[:],
                            )
                        for k in range(4):
                            nc.any.tensor_copy(
                                out=at[4 * g + k][:, col:col + P],
                                in_=pst[:, k * P:(k + 1) * P],
                            )

                def emit_outproj(b4, rt):
                    for gcol in range(H // 512):
                        ps_o = [pp.tile([P, 512], dt.float32, tag="ps",
                                        name="ps") for _ in range(4)]
                        for hb in range(HC):
                            wo_t = wopool.tile([P, 512], BFO, tag="wo",
                                               name="wo")
                            nc.sync.dma_start(
                                wo_t[:],
                                wo[hb * P:(hb + 1) * P,
                                   gcol * 512:(gcol + 1) * 512],
                            )
                            for isub in range(4):
                                nc.tensor.matmul(
                                    ps_o[isub][:],
                                    rt[hb][:, isub * P:(isub + 1) * P],
                                    wo_t[:],
                                    start=(hb == 0),
                                    stop=(hb == HC - 1),
                                )
                        for isub in range(4):
                            og = ogpool.tile([P, 512], dt.float32, tag="og",
                                             name="og")
                            if bo_sb is not None:
                                nc.vector.tensor_tensor(
                                    og[:], ps_o[isub][:],
                                    bo_sb[:, gcol * 512:(gcol + 1) * 512],
                                    OP.add,
                                )
                                nc.scalar.activation(og[:], og[:], GELU)
                            else:
                                nc.scalar.activation(og[:], ps_o[isub][:], GELU)
                            row = (4 * b4 + isub) * P
                            nc.sync.dma_start(
                                ret_out[row:row + P,
                                        gcol * 512:(gcol + 1) * 512],
                                og[:],
                            )

                pending_outproj = None  # deferred to fill PE gaps
                for b4 in range(NB if PHASE >= 1 else 0):
                    Jb = 512 * (b4 + 1)
                    njt = b4 + 1
                    njc = 4 * (b4 + 1)
                    at = [atpool.tile([P, 512], BF, tag="at",
                                      name="at") for _ in range(njc)]
                    s_tiles = {}

                    prev = None
                    for bi in range(4 * b4, 4 * b4 + 4):
                        emit_scores(bi, njt, Jb, s_tiles)
                        if b4 == 0 and bi == 0:
                            load_wv()
                        if prev is not None and PHASE >= 2:
                            emit_transposes(prev, njt, s_tiles, at)
                        prev = bi
                    # fill the softmax tail with the previous block's
                    # out-projection and this block's v tiles
                    if pending_outproj is not None:
                        emit_outproj(*pending_outproj)
                        pending_outproj = None
                    for jb in range(4 * b4, 4 * b4 + 4):
                        emit_v(jb)
                    if PHASE >= 2:
                        emit_transposes(prev, njt, s_tiles, at)
                    if PHASE < 3:
                        continue

                    # ret_preT[h, i] = sum_j v[j, h] * attnT[j, i]
                    rt = [rtpool.tile([P, 512], BFO, tag="rt",
                                      name="rt") for _ in range(HC)]
                    for hb in range(HC):
                        ps = pp.tile([P, 512], dt.float32, tag="ps", name="ps")
                        for jc in range(njc):
                            nc.tensor.matmul(
                                ps[:],
                                v_sb[jc][:, hb * P:(hb + 1) * P],
                                at[jc][:],
                                start=(jc == 0),
                                stop=(jc == njc - 1),
                            )
                        nc.any.tensor_copy(out=rt[hb][:], in_=ps[:])
                    if NODEFER:
                        emit_outproj(b4, rt)
                    else:
                        pending_outproj = (b4, rt)

                if pending_outproj is not None:
                    emit_outproj(*pending_outproj)
                if tmp0pool is not None:
                    tmp0pool.__exit__(None, None, None)

    nc.finalize()
    return nc


def _make_runner(nc):
    """Build a cached multi-core PJRT runner for a finalized Bass program.

    Mirrors concourse.bass2jax.run_bass_via_pjrt's multi-core path, but keeps
    the jitted callable so repeated invocations don't re-trace/re-compile.
    """
    import jax
    import concourse.mybir as mybir
    from jax.experimental.shard_map import shard_map
    from jax.sharding import Mesh, PartitionSpec
    from concourse import bass2jax as b2j

    b2j.install_neuronx_cc_hook()

    partition_name = (
        nc.partition_id_tensor.name if nc.partition_id_tensor else None
    )
    in_names, out_names, out_avals, zero_shapes = [], [], [], []
    for alloc in nc.m.functions[0].allocations:
        if not isinstance(alloc, mybir.MemoryLocationSet):
            continue
        name = alloc.memorylocations[0].name
        if alloc.kind == "ExternalInput":
            if name != partition_name:
                in_names.append(name)
        elif alloc.kind == "ExternalOutput":
            shape = tuple(alloc.tensor_shape)
            dtype = mybir.dt.np(alloc.dtype)
            out_names.append(name)
            out_avals.append(jax.core.ShapedArray(shape, dtype))
            zero_shapes.append((shape, dtype))
    n_params = len(in_names)
    n_outs = len(out_avals)
    all_in_names = list(in_names) + list(out_names)
    if partition_name is not None:
        all_in_names.append(partition_name)
    donate = tuple(range(n_params, n_params + n_outs))

    def _body(*args):
        operands = list(args)
        if partition_name is not None:
            operands.append(b2j.partition_id_tensor())
        outs = b2j._bass_exec_p.bind(
            *operands,
            out_avals=tuple(out_avals),
            in_names=tuple(all_in_names),
            out_names=tuple(out_names),
            lowering_input_output_aliases=(),
            sim_require_finite=True,
            sim_require_nnan=True,
            nc=nc,
        )
        return tuple(outs)

    devices = jax.devices()[:N_CORES]
    mesh = Mesh(np.asarray(devices), ("core",))
    in_specs = (PartitionSpec("core"),) * (n_params + n_outs)
    out_specs = (PartitionSpec("core"),) * n_outs
    sharded = jax.jit(
        shard_map(
            _body, mesh=mesh, in_specs=in_specs, out_specs=out_specs,
            check_rep=False,
        ),
        donate_argnums=donate,
        keep_unused=True,
    )

    def make_zeros():
        return [
            np.zeros((N_CORES * shp[0], *shp[1:]), dtp)
            for shp, dtp in zero_shapes
        ]

    def concat_inputs(in_maps):
        return [
            np.concatenate([np.asarray(in_maps[c][k]) for c in range(N_CORES)],
                           axis=0)
            for k in in_names
        ]

    def run(in_maps):
        out_arrs = sharded(*concat_inputs(in_maps), *make_zeros())
        return [
            {
                k: np.asarray(out_arrs[i]).reshape(
                    N_CORES, *out_avals[i].shape
                )[c]
                for i, k in enumerate(out_names)
            }
            for c in range(N_CORES)
        ]

    run.sharded = sharded
    run.concat_inputs = concat_inputs
    run.make_zeros = make_zeros
    run.out_names = out_names
    run.out_avals = out_avals
    return run


def _prepare_in_maps(query_state, memory_state, time_seq, Wq, bq, Wk, bk,
                     Wv, bv, Wo, bo, order_scale, time_scale, padding_mask):
    import ml_dtypes

    bf = np.float32 if os.environ.get("K_F32RET") else ml_dtypes.bfloat16
    bfo = np.float32 if os.environ.get("K_F32OP") else ml_dtypes.bfloat16
    q = np.asarray(query_state, dtype=np.float32)
    m = np.asarray(memory_state, dtype=np.float32)
    t = np.asarray(time_seq, dtype=np.float32)
    pad = np.asarray(padding_mask).astype(bool)
    Wq = np.ascontiguousarray(np.asarray(Wq, dtype=np.float32))
    Wk = np.ascontiguousarray(np.asarray(Wk, dtype=np.float32))
    Wv16 = np.ascontiguousarray(
        np.asarray(Wv, dtype=np.float32).astype(bf)
    )
    Wo16 = np.ascontiguousarray(
        np.asarray(Wo, dtype=np.float32).astype(bfo)
    )
    bq = np.asarray(bq, dtype=np.float32)
    bk = np.asarray(bk, dtype=np.float32)
    bv = np.asarray(bv, dtype=np.float32)
    bo = np.asarray(bo, dtype=np.float32)

    def softplus(x):
        x = float(np.asarray(x))
        return float(np.log1p(np.exp(-abs(x))) + max(x, 0.0))

    co = np.float32(-softplus(order_scale))
    ct = np.float32(-softplus(time_scale))

    idx = np.arange(S, dtype=np.float32)
    od = np.maximum(idx[:, None] - idx[None, :], 0.0)
    ob = co * np.log1p(od)  # [S,S] f32
    tri = np.triu(np.ones((S, S), dtype=bool))  # strictly-causal invalid

    flags = (bool(np.any(bq)), bool(np.any(bk)),
             bool(np.any(bv)), bool(np.any(bo)))

    in_maps = []
    for b in range(B):
        tg = np.maximum(t[b][:, None] - t[b][None, :], 0.0)
        bias_b = ob + ct * np.log1p(tg)
        invalid = tri | pad[b][None, :]
        bias_b[invalid] = NEG
        # blocked MT copy: mtb[jb, p, dc*128 + j] = M[jb*128+j, dc*128+p]
        mtb = np.ascontiguousarray(
            m[b].astype(bf)
            .reshape(NJB, P, DC, P).transpose(0, 3, 2, 1)
            .reshape(NJB, P, H)
        )
        im = {
            "xt": np.ascontiguousarray(q[b].T),
            "mt": np.ascontiguousarray(m[b].T),
            "mtb": mtb,
            "wq": Wq, "wk": Wk, "wv": Wv16, "wo": Wo16,
            "bias": np.ascontiguousarray(bias_b, dtype=np.float32),
        }
        if flags[0]:
            im["bq"] = np.ascontiguousarray(bq.reshape(HC, P).T)
        if flags[1]:
            im["bk"] = np.ascontiguousarray(bk.reshape(HC, P).T)
        if flags[2]:
            im["bv"] = np.ascontiguousarray(np.broadcast_to(bv, (P, H)))
        if flags[3]:
            im["bo"] = np.ascontiguousarray(np.broadcast_to(bo, (P, H)))
        in_maps.append(im)
    return in_maps, flags


def _get_runner(flags):
    global _RUNNER
    if _RUNNER is None or _RUNNER[1] != flags:
        nc = _build_nc(*flags)
        _RUNNER = (_make_runner(nc), flags)
    return _RUNNER[0]


def kernel(**inputs):
    in_maps, flags = _prepare_in_maps(**inputs)
    run = _get_runner(flags)
    res = run(in_maps)
    retrieved = np.stack([res[c]["ret"] for c in range(N_CORES)])
    attn = np.stack([res[c]["attn"] for c in range(N_CORES)])
    return retrieved, attn
